# revision 1
# baseline (speedup 1.0000x reference)
"""Trainium2 Bass kernel for nn_MixedAttention.

Full inputs in, full output out. Sharding: 8 cores = 2 (batch) x 4 (head
pairs). Each core computes 2 global + 2 local heads for one batch element.

Key algebraic rewrite for the local branch:
    lscores = (lq@lk1^T)@(lk1@lk2^T) = lq @ (lk1^T@lk1) @ lk2^T
with M = lk1^T@lk1 a [64,64] matrix -- turns a 2048^3 matmul chain into
two small matmuls plus one S x S matmul (30x less PE work).

Precision/dtype strategy: fp32 matmuls run at ~2 cycles/column on the PE,
float32r (TF32-like, ~13-bit mantissa) at ~1. The local-branch scores are
large (|raw| up to ~2000) and feed exp(), so any input rounding there is
amplified exponentially -> the local score chain (hidden^T, score-side
projections, pass-2 score matmul) stays fp32. Everything whose error is
not exp-amplified runs f32r: global q/k (scores |s|<~5), all value paths,
the probs @ v context matmuls, and the local pass-1 max (only needs ~1 ulp
of exp range).

Layout: scores are computed transposed st[j, i] = K_eff @ Q_eff^T so the
context matmul needs no transposed probs (lhsT = v_nat, rhs = e). v gets
an extra ones column so the softmax denominator falls out of the context
matmul for free. Global heads skip max subtraction entirely (mask folded
into the Exp bias); local heads get an exact row max from a separate
f32r pass in the untransposed orientation (free-dim reduce_max), and the
-max correction rides an extra contraction row (K=65) in pass 2.
"""

import math
import os
import sys

import numpy as np

sys.path.insert(0, "/opt/trn_rl_repo")

B, S, HID, HEAD = 2, 2048, 1024, 64
SC = S // 128  # 16 s-chunks of 128
HC = HID // 128  # 8 hidden chunks
N_CORES = 8
SCALE = 1.0 / math.sqrt(HEAD)

W_NAMES = ["wq", "wk", "wv", "wlq", "wlk1", "wlk2", "wlv"]
F32R_PROJ = {"wq", "wk", "wv"}  # projections written as f32r at the source

_CACHE = {}
LAST_RESULTS = None  # stash of BassKernelResults for test.py profiling


def _build():
    import concourse.mybir as mybir
    import concourse.tile as tile
    from concourse import bacc
    from concourse.masks import make_identity

    f32 = mybir.dt.float32
    f32r = mybir.dt.float32r
    AF = mybir.ActivationFunctionType
    ALU = mybir.AluOpType
    AX = mybir.AxisListType

    nc = bacc.Bacc("TRN2", target_bir_lowering=False, debug=False,
                   enable_asserts=False)

    hid_d = nc.dram_tensor("hid", (HID, S), f32, kind="ExternalInput").ap()
    mask_d = nc.dram_tensor("mask", (S,), f32, kind="ExternalInput").ap()
    w_d = {n: nc.dram_tensor(n, (HID, 128), f32, kind="ExternalInput").ap()
           for n in W_NAMES}
    b_d = {n: nc.dram_tensor("b" + n[1:], (128,), f32,
                             kind="ExternalInput").ap() for n in W_NAMES}
    out_d = nc.dram_tensor("out", (S, 256), f32, kind="ExternalOutput").ap()

    with tile.TileContext(nc) as tc:
        with (
            tc.tile_pool(name="const", bufs=1) as constp,
            tc.tile_pool(name="persist", bufs=1) as pp,
            tc.tile_pool(name="wp_g", bufs=1) as wp_g,
            tc.tile_pool(name="epool", bufs=8) as ep,
            tc.tile_pool(name="opool", bufs=1) as op_,
            tc.tile_pool(name="ps_tr", bufs=2, space="PSUM") as ps_tr,
            tc.tile_pool(name="ps_mm", bufs=4, space="PSUM") as ps_mm,
            tc.tile_pool(name="ps_ctx", bufs=2, space="PSUM") as ps_ctx,
            tc.tile_pool(name="dramp", bufs=2, space="DRAM") as dramp,
        ):
            ident = constp.tile([128, 128], f32, name="ident")
            make_identity(nc, ident)
            identr = constp.tile([128, 128], f32r, name="identr")
            nc.vector.tensor_copy(identr, ident)
            ones_sb = constp.tile([128, SC], f32, name="ones_sb")
            nc.vector.memset(ones_sb, 1.0)
            mask_sb = constp.tile([128, SC], f32, name="mask_sb")
            nc.gpsimd.dma_start(mask_sb,
                                mask_d.rearrange("(c p) -> p c", p=128))
            bias_sb = {}
            for n in W_NAMES:
                t = constp.tile([128, 1], f32, name=f"b_{n}")
                nc.gpsimd.dma_start(t, b_d[n][:, None])
                bias_sb[n] = t

            projT = {n: pp.tile([128, S], f32, name=f"projT_{n}")
                     for n in W_NAMES if n not in F32R_PROJ}

            out_sb = op_.tile([128, SC, 256], f32, name="out_sb")

            # ---------- emission helpers ----------

            def emit_wdma(n, iop):
                wsb = iop.tile([128, HC, 128], f32, tag="w", name=f"w_{n}")
                nc.sync.dma_start(
                    wsb, w_d[n].rearrange("(c p) m -> p c m", p=128))
                return wsb

            def emit_proj_half(n, wsb, hidT, half):
                accs = [ps_mm.tile([128, 512], f32, tag="mm",
                                   name=f"acc{i}") for i in range(2)]
                for hc in range(HC):
                    for ic in range(2):
                        icg = half * 2 + ic
                        nc.tensor.matmul(
                            accs[ic], lhsT=wsb[:, hc],
                            rhs=hidT[:, hc, icg * 512:(icg + 1) * 512],
                            start=(hc == 0), stop=(hc == HC - 1))
                for ic in range(2):
                    icg = half * 2 + ic
                    nc.vector.tensor_scalar_add(
                        projT[n][:, icg * 512:(icg + 1) * 512],
                        accs[ic], bias_sb[n])

            def emit_proj(n, hidT, iop):
                wsb = emit_wdma(n, iop)
                for half in range(2):
                    emit_proj_half(n, wsb, hidT, half)

            def build_vaug(vT, vdt):
                # v natural [s, d] + ones column -> [128, SC, 65] f32r
                idm = identr if vdt == f32r else ident
                base = vT.base_partition()
                idsl = slice(base, base + 64)
                vaug = wp_g.tile([128, SC, 65], f32r, tag="vaug",
                                 name="vaug", bufs=2)
                nc.vector.tensor_copy(vaug[:, :, 64], ones_sb)
                for t in range(SC):
                    pt = ps_tr.tile([128, 128], vdt, tag="tr")
                    nc.tensor.transpose(
                        pt[:, :64], vT[:, t * 128:(t + 1) * 128],
                        idm[idsl, idsl])
                    nc.any.tensor_copy(vaug[:, t, :64], pt[:, :64])
                return vaug

            def attention_ic(head, kT, qT, vaug, is_local, ic):
                # main pass: st -> exp -> ctx (+sums via the ones column),
                # then transpose back and divide by the sums
                csl = slice(head * 64, (head + 1) * 64)
                if True:
                    isl = slice(ic * 512, (ic + 1) * 512)
                    ctx = ps_ctx.tile([65, 512], f32, tag="ctx", name="ctx")

                    def ctx_group(es):
                        for jc, e in es:
                            nc.tensor.matmul(ctx, lhsT=vaug[:, jc], rhs=e,
                                             start=(jc == 0),
                                             stop=(jc == SC - 1))

                    prev = None
                    for jg in range(4):
                        es = []
                        for jj in range(4):
                            jc = jg * 4 + jj
                            jsl = slice(jc * 128, (jc + 1) * 128)
                            st = ps_mm.tile([128, 512], f32, tag="mm",
                                            name="st")
                            nc.tensor.matmul(st, lhsT=kT[:, jsl],
                                             rhs=qT[:, isl],
                                             start=True, stop=True)
                            e = ep.tile([128, 512], f32r, tag="e", name="e")
                            bias = 0.0 if is_local else mask_sb[:, jc:jc + 1]
                            nc.scalar.activation(e, st, AF.Exp, bias=bias,
                                                 scale=SCALE)
                            es.append((jc, e))
                        if prev is not None:
                            ctx_group(prev)
                        prev = es
                    ctx_group(prev)
                    ctx_sbc = wp_g.tile([65, 512], f32, tag="ctx_sbc",
                                        name="ctx_sbc")
                    nc.any.tensor_copy(ctx_sbc, ctx)
                    for tt in range(4):
                        t = ic * 4 + tt
                        pt = ps_tr.tile([128, 128], f32, tag="tr")
                        nc.tensor.transpose(
                            pt[:, :65], ctx_sbc[:, tt * 128:(tt + 1) * 128],
                            ident[:65, :65])
                        rec = wp_g.tile([128, 1], f32, tag="rec", name="rec")
                        nc.vector.reciprocal(rec, pt[:, 64:65])
                        nc.vector.tensor_scalar_mul(
                            out_sb[:, t, csl], pt[:, :64], rec)
                    nc.sync.dma_start(
                        out_d.rearrange("(t p) c -> p t c", p=128)[
                            :, ic * 4:(ic + 1) * 4, csl],
                        out_sb[:, ic * 4:(ic + 1) * 4, csl])

            def local_prep(head, wp):
                hh = head % 2
                rs = slice(hh * 64, (hh + 1) * 64)
                if hh == 0:
                    lqT = projT["wlq"][rs]
                    lk1T = projT["wlk1"][rs]
                else:
                    lqT = wp.tile([64, S], f32, tag="s0l", name="s0l")
                    nc.scalar.copy(lqT, projT["wlq"][rs])
                    lk1T = wp.tile([64, S], f32, tag="s1l", name="s1l")
                    nc.scalar.copy(lk1T, projT["wlk1"][rs])

                # lk1 natural [s, d] via transposes
                lk1nat = wp.tile([128, SC, 64], f32, tag="lk1nat",
                                 name="lk1nat", bufs=2)
                for t in range(SC):
                    pt = ps_tr.tile([128, 128], f32, tag="tr")
                    nc.tensor.transpose(
                        pt[:, :64], lk1T[:, t * 128:(t + 1) * 128],
                        ident[:64, :64])
                    nc.any.tensor_copy(lk1nat[:, t], pt[:, :64])
                # M = lk1^T @ lk1 [64, 64] (symmetric)
                mps = ps_mm.tile([128, 512], f32, tag="mm", name="mps")
                for t in range(SC):
                    nc.tensor.matmul(mps[:64, :64], lhsT=lk1nat[:, t],
                                     rhs=lk1nat[:, t],
                                     start=(t == 0), stop=(t == SC - 1))
                m_sb = wp.tile([64, 64], f32, tag="m_sb", name="m_sb",
                               bufs=2)
                nc.any.tensor_copy(m_sb, mps[:64, :64])
                # qaug rows 0:64 = (lq @ M)^T = M @ lq^T (M symmetric);
                # row 64 filled later with -max
                qaug = wp.tile([65, S], f32, tag="qaug", name="qaug",
                               bufs=2)
                for ic in range(4):
                    mm = ps_mm.tile([128, 512], f32, tag="mm", name="mm")
                    nc.tensor.matmul(mm[:64], lhsT=m_sb,
                                     rhs=lqT[:, ic * 512:(ic + 1) * 512],
                                     start=True, stop=True)
                    nc.any.tensor_copy(qaug[:64, ic * 512:(ic + 1) * 512],
                                       mm[:64])
                # k2aug: rows 0:64 = lk2^T, row 64 = ones
                k2aug = wp.tile([65, S], f32, tag="k2aug", name="k2aug",
                                bufs=2)
                nc.scalar.copy(k2aug[:64, :], projT["wlk2"][rs])
                nc.vector.memset(k2aug[64:65, :], 1.0)
                vaug = build_vaug(projT["wlv"][rs], f32)

                # f32r shadows for pass 1 (max only needs ~1 absolute)
                qaug_r = wp.tile([64, S], f32r, tag="qaug_r", name="qaug_r",
                                 bufs=2)
                nc.scalar.copy(qaug_r, qaug[:64])
                k2aug_r = wp.tile([64, S], f32r, tag="k2aug_r",
                                  name="k2aug_r", bufs=2)
                nc.scalar.copy(k2aug_r, k2aug[:64])
                return dict(qaug=qaug, k2aug=k2aug, vaug=vaug,
                            qaug_r=qaug_r, k2aug_r=k2aug_r)

            def local_pass1(head, hs, wp):
                # pass 1: untransposed s[i, j] blocks; row max via free-dim
                # reduce (independent ops; no serial DVE chain)
                qaug_r, k2aug_r = hs["qaug_r"], hs["k2aug_r"]
                maxneg = wp.tile([128, SC], f32, tag="maxneg", name="maxneg",
                                 bufs=2)
                for t in range(SC):
                    pmax = wp.tile([128, 4], f32, tag="pmax", name="pmax",
                                   bufs=2)
                    for j4 in range(4):
                        st = ps_mm.tile([128, 512], f32, tag="mm", name="st1")
                        nc.tensor.matmul(
                            st, lhsT=qaug_r[:, t * 128:(t + 1) * 128],
                            rhs=k2aug_r[:, j4 * 512:(j4 + 1) * 512],
                            start=True, stop=True)
                        nc.vector.tensor_reduce(pmax[:, j4:j4 + 1], st,
                                                axis=AX.X, op=ALU.max)
                    nc.vector.tensor_reduce(maxneg[:, t:t + 1], pmax,
                                            axis=AX.X, op=ALU.max,
                                            negate=True)
                mscr = dramp.tile([S], f32, tag="mscr", name="mscr")
                nc.sync.dma_start(
                    mscr.rearrange("(t p) -> p t", p=128), maxneg)
                nc.sync.dma_start(hs["qaug"][64:65, :], mscr[None, :])

                        # ---------- phase A: hidden^T, projections, global heads ----
            with (
                tc.tile_pool(name="pp_g", bufs=1) as pp_g,
                tc.tile_pool(name="hidT", bufs=1) as hp,
                tc.tile_pool(name="io", bufs=2) as iop,
            ):
                for n in F32R_PROJ:
                    projT[n] = pp_g.tile([128, S], f32r, name=f"projT_{n}")
                hidT = hp.tile([128, HC, S], f32, name="hidT")
                hid_r = hid_d.rearrange("(c p) s -> p c s", p=128)
                # qkv weights first on the gpsimd queue so the first
                # projection matmuls start as soon as hidT chunk 0 lands
                wsb_g = {}
                for n in ["wq", "wk", "wv"]:
                    wsb_g[n] = iop.tile([128, HC, 128], f32, tag="wg",
                                        name=f"w_{n}")
                    nc.gpsimd.dma_start(
                        wsb_g[n], w_d[n].rearrange("(c p) m -> p c m", p=128))
                for hc in range(HC):
                    eng = nc.sync if hc % 2 == 0 else nc.gpsimd
                    eng.dma_start(hidT[:, hc], hid_r[:, hc])
                for n in ["wq", "wk", "wv"]:
                    for half in range(2):
                        emit_proj_half(n, wsb_g[n], hidT, half)
                gvaug = {}
                for hh in range(2):
                    rs = slice(hh * 64, (hh + 1) * 64)
                    gvaug[hh] = build_vaug(projT["wv"][rs], f32r)
                # interleave: global-head attention units between local
                # projection halves so the in-order PE queue always has
                # independent matmuls (keeps HAM warm)
                lp = [(n, half) for n in ["wlq", "wlk1", "wlk2", "wlv"]
                      for half in range(2)]
                wsbs = {}
                for i, (hh, ic) in enumerate(
                        [(h, c) for h in range(2) for c in range(4)]):
                    rs = slice(hh * 64, (hh + 1) * 64)
                    attention_ic(hh, projT["wk"][rs], projT["wq"][rs],
                                 gvaug[hh], False, ic)
                    n, half = lp[i]
                    if half == 0:
                        wsbs[n] = emit_wdma(n, iop)
                    emit_proj_half(n, wsbs[n], hidT, half)

            # ---------- phase B: local heads (stage-interleaved so
            # the PE never idles long enough to go HAM-cold) ----------
            with tc.tile_pool(name="wp_l", bufs=1) as wp_l:
                st2 = local_prep(2, wp_l)
                st3 = local_prep(3, wp_l)
                local_pass1(2, st2, wp_l)
                local_pass1(3, st3, wp_l)
                for ic in range(4):
                    attention_ic(2, st2["k2aug"], st2["qaug"], st2["vaug"],
                                 True, ic)
                for ic in range(4):
                    attention_ic(3, st3["k2aug"], st3["qaug"], st3["vaug"],
                                 True, ic)

    nc.compile()
    return nc


def _patch_ldw_opt():
    # walrus ships with the LDWEIGHTS optimizer disabled; fp32 matmuls
    # pay a bundled weight reload per matmul, so try enabling the
    # optimizer (verified against the reference output by the caller).
    from concourse import bass_utils
    if getattr(bass_utils, "_ldw_patched", False):
        return
    orig = bass_utils.bir_verify_and_optimise

    def patched(*a, **k):
        import subprocess
        orig_run = bass_utils.run_command

        def run2(cmd, **kw):
            cmd = [c.replace("--enable-ldw-opt=false",
                             "--enable-ldw-opt=true") for c in cmd]
            return orig_run(cmd, **kw)

        bass_utils.run_command = run2
        try:
            return orig(*a, **k)
        finally:
            bass_utils.run_command = orig_run

    bass_utils.bir_verify_and_optimise = patched
    bass_utils._ldw_patched = True


def kernel(**inputs):
    from concourse import bass_utils

    if os.environ.get("LDW_OPT", "0") == "1":
        _patch_ldw_opt()

    global LAST_RESULTS
    if "nc" not in _CACHE:
        _CACHE["nc"] = _build()
    nc = _CACHE["nc"]

    inputs = dict(inputs)
    inputs["wlv"] = np.asarray(inputs["wlv1"]) + np.asarray(inputs["wlv2"])
    inputs["blv"] = np.asarray(inputs["blv1"]) + np.asarray(inputs["blv2"])
    hs = np.ascontiguousarray(np.asarray(inputs["hidden_states"], np.float32))
    am = np.ascontiguousarray(np.asarray(inputs["attention_mask"], np.float32))
    in_maps = []
    for c in range(N_CORES):
        b, g = c // 4, c % 4
        csl = slice(128 * g, 128 * (g + 1))
        m = {"hid": np.ascontiguousarray(hs[b].T), "mask": am[b, 0, 0]}
        for n in W_NAMES:
            m[n] = np.ascontiguousarray(
                np.asarray(inputs[n], np.float32)[:, csl])
            m["b" + n[1:]] = np.ascontiguousarray(
                np.asarray(inputs["b" + n[1:]], np.float32)[csl])
        in_maps.append(m)

    res = bass_utils.run_bass_kernel_spmd(
        nc, in_maps, list(range(N_CORES)),
        tmpdir=os.environ.get("BASS_TMPDIR"))
    LAST_RESULTS = res

    out = np.zeros((B, S, HID), np.float32)
    for c in range(N_CORES):
        b, g = c // 4, c % 4
        o = res.results[c]["out"]
        out[b, :, 128 * g:128 * (g + 1)] = o[:, :128]
        out[b, :, 512 + 128 * g:512 + 128 * (g + 1)] = o[:, 128:]
    return out



# revision 7
# speedup vs baseline: 1.6575x; 1.6575x over previous
"""Trainium2 Bass kernel for nn_MixedAttention.

Full inputs in, full output out. Sharding: 8 cores = 2 (batch) x 4 (head
pairs). Each core computes 2 global + 2 local heads for one batch element.

Key algebraic rewrite for the local branch:
    lscores = (lq@lk1^T)@(lk1@lk2^T) = lq @ (lk1^T@lk1) @ lk2^T
with M = lk1^T@lk1 a [64,64] matrix -- turns a 2048^3 matmul chain into
two small matmuls plus one S x S matmul (30x less PE work).

Precision strategy (validated against a numpy bit-exact simulation of
f32r = round-to-nearest @ 11 explicit mantissa bits, measured on HW):
everything runs f32r (1 cyc/row on the PE vs 4 for fp32), except the
small M matmul. The exp/value/context path runs bf16 (cheap weight
loads); global q/k are written bf16 straight from the projection. The
sim puts this config at ~7.4e-3 rel err vs the 2e-2 gate.

Layout: scores are computed transposed st[j, i] = K_eff @ Q_eff^T so the
context matmul needs no transposed probs (lhsT = v_nat, rhs = e). v gets
an extra ones column so the softmax denominator falls out of the context
matmul for free. Global heads skip max subtraction entirely (mask folded
into the Exp bias); local heads get a row max estimate from a separate
f32r pass in the untransposed orientation (free-dim reduce_max), and the
-max correction rides an extra contraction row (K=65) in pass 2.
"""

import math
import os
import sys

import numpy as np

sys.path.insert(0, "/opt/trn_rl_repo")

B, S, HID, HEAD = 2, 2048, 1024, 64
SC = S // 128  # 16 s-chunks of 128
HC = HID // 128  # 8 hidden chunks
N_CORES = 8
SCALE = 1.0 / math.sqrt(HEAD)

W_NAMES = ["wq", "wk", "wv", "wlq", "wlk1", "wlk2", "wlv"]

_CACHE = {}
LAST_RESULTS = None  # stash of BassKernelResults for test.py profiling


def _build():
    import concourse.mybir as mybir
    import concourse.tile as tile
    from concourse import bacc
    from concourse.masks import make_identity

    f32 = mybir.dt.float32
    f32r = mybir.dt.float32r
    bf16 = mybir.dt.bfloat16
    AF = mybir.ActivationFunctionType
    ALU = mybir.AluOpType
    AX = mybir.AxisListType

    nc = bacc.Bacc("TRN2", target_bir_lowering=False, debug=False,
                   enable_asserts=False)

    hid_d = nc.dram_tensor("hid", (HID, S), f32r, kind="ExternalInput").ap()
    mask_d = nc.dram_tensor("mask", (S,), f32, kind="ExternalInput").ap()
    w_d = {n: nc.dram_tensor(n, (HID, 128), f32r, kind="ExternalInput").ap()
           for n in W_NAMES}
    b_d = {n: nc.dram_tensor("b" + n[1:], (128,), f32,
                             kind="ExternalInput").ap() for n in W_NAMES}
    out_d = nc.dram_tensor("out", (S, 256), f32, kind="ExternalOutput").ap()

    # projT storage dtype per projection: global q/k and both value mats
    # are only read by bf16 consumers; wlq/wlk1 stay f32 (score chain),
    # read as f32r via bitcast. wlk2 goes straight into the k2aug tiles.
    PROJ_BF16 = {"wq", "wk", "wv", "wlv"}

    with tile.TileContext(nc) as tc:
        with (
            tc.tile_pool(name="const", bufs=1) as constp,
            tc.tile_pool(name="persist", bufs=1) as pp,
            tc.tile_pool(name="wp_g", bufs=1) as wp_g,
            tc.tile_pool(name="epool", bufs=8) as ep,
            tc.tile_pool(name="opool", bufs=1) as op_,
            tc.tile_pool(name="ps_tr", bufs=2, space="PSUM") as ps_tr,
            tc.tile_pool(name="ps_mm", bufs=4, space="PSUM") as ps_mm,
            tc.tile_pool(name="ps_ctx", bufs=2, space="PSUM") as ps_ctx,
            tc.tile_pool(name="dramp", bufs=2, space="DRAM") as dramp,
        ):
            ident = constp.tile([128, 128], f32, name="ident")
            make_identity(nc, ident)
            identb = constp.tile([128, 128], bf16, name="identb")
            nc.vector.tensor_copy(identb, ident)
            ones_sb = constp.tile([128, SC], bf16, name="ones_sb")
            nc.vector.memset(ones_sb, 1.0)
            mask_sb = constp.tile([128, SC], f32, name="mask_sb")
            nc.gpsimd.dma_start(mask_sb,
                                mask_d.rearrange("(c p) -> p c", p=128))
            bias_sb = {}
            for n in W_NAMES:
                t = constp.tile([128, 1], f32, name=f"b_{n}")
                nc.gpsimd.dma_start(t, b_d[n][:, None])
                bias_sb[n] = t

            projT = {"wlq": pp.tile([128, S], f32r, name="projT_wlq"),
                     "wlk1": pp.tile([128, S], f32, name="projT_wlk1")}
            for n in PROJ_BF16:
                projT[n] = pp.tile([128, S], bf16, name=f"projT_{n}")
            # k2aug per local head: rows 0:64 = lk2^T (written by the
            # projection directly), row 64 = ones
            k2aug = [pp.tile([65, S], f32r, name=f"k2aug_{hh}")
                     for hh in range(2)]
            ones_row = constp.tile([1, S], f32, name="ones_row")
            nc.vector.memset(ones_row, 1.0)
            for hh in range(2):
                nc.vector.tensor_copy(k2aug[hh][64:65, :], ones_row)

            out_sb = op_.tile([128, SC, 256], f32, name="out_sb")

            # ---------- emission helpers ----------

            def emit_wdma(n, iop):
                wsb = iop.tile([128, HC, 128], f32r, tag="w", name=f"w_{n}")
                nc.sync.dma_start(
                    wsb, w_d[n].rearrange("(c p) m -> p c m", p=128))
                return wsb

            def proj_out(n, icg):
                # destination AP(s) for projection column group icg
                isl = slice(icg * 512, (icg + 1) * 512)
                if n == "wlk2":
                    return [(k2aug[0][:64, isl], slice(0, 64)),
                            (k2aug[1][:64, isl], slice(64, 128))]
                return [(projT[n][:, isl], slice(0, 128))]

            def emit_proj_half(n, wsb, hidT, half):
                accs = [ps_mm.tile([128, 512], f32, tag="mm",
                                   name=f"acc{i}") for i in range(2)]
                for hc in range(HC):
                    for ic in range(2):
                        icg = half * 2 + ic
                        nc.tensor.matmul(
                            accs[ic], lhsT=wsb[:, hc],
                            rhs=hidT[:, hc, icg * 512:(icg + 1) * 512],
                            start=(hc == 0), stop=(hc == HC - 1))
                for ic in range(2):
                    icg = half * 2 + ic
                    for dst, rs in proj_out(n, icg):
                        nc.vector.tensor_scalar_add(
                            dst, accs[ic][rs], bias_sb[n][rs])

            def build_vaug(vT, wp):
                # v natural [s, d] + ones column -> [128, SC, 65] bf16
                base = vT.base_partition()
                idsl = slice(base, base + 64)
                vaug = wp.tile([128, SC, 65], bf16, tag="vaug",
                               name="vaug", bufs=2)
                nc.vector.tensor_copy(vaug[:, :, 64], ones_sb)
                for t in range(SC):
                    pt = ps_tr.tile([128, 128], bf16, tag="tr")
                    nc.tensor.transpose(
                        pt[:, :64], vT[:, t * 128:(t + 1) * 128],
                        identb[idsl, idsl])
                    nc.any.tensor_copy(vaug[:, t, :64], pt[:, :64])
                return vaug

            def attention_ic(head, kT, qT, vaug, is_local, ic):
                # main pass: st -> exp -> ctx (+sums via the ones column),
                # then transpose back and divide by the sums
                csl = slice(head * 64, (head + 1) * 64)
                isl = slice(ic * 512, (ic + 1) * 512)
                ctx = ps_ctx.tile([65, 512], f32, tag="ctx", name="ctx")

                def ctx_group(es):
                    for jc, e in es:
                        nc.tensor.matmul(ctx, lhsT=vaug[:, jc], rhs=e,
                                         start=(jc == 0),
                                         stop=(jc == SC - 1))

                prev = None
                for jg in range(4):
                    es = []
                    for jj in range(4):
                        jc = jg * 4 + jj
                        jsl = slice(jc * 128, (jc + 1) * 128)
                        st = ps_mm.tile([128, 512], f32, tag="mm",
                                        name="st")
                        nc.tensor.matmul(st, lhsT=kT[:, jsl],
                                         rhs=qT[:, isl],
                                         start=True, stop=True)
                        e = ep.tile([128, 512], bf16, tag="e", name="e")
                        bias = 0.0 if is_local else mask_sb[:, jc:jc + 1]
                        nc.scalar.activation(e, st, AF.Exp, bias=bias,
                                             scale=SCALE)
                        es.append((jc, e))
                    if prev is not None:
                        ctx_group(prev)
                    prev = es
                ctx_group(prev)
                ctx_sbc = wp_g.tile([65, 512], bf16, tag="ctx_sbc",
                                    name="ctx_sbc")
                nc.any.tensor_copy(ctx_sbc, ctx)
                for tt in range(4):
                    t = ic * 4 + tt
                    pt = ps_tr.tile([128, 128], bf16, tag="tr")
                    nc.tensor.transpose(
                        pt[:, :65], ctx_sbc[:, tt * 128:(tt + 1) * 128],
                        identb[:65, :65])
                    rec = wp_g.tile([128, 1], f32, tag="rec", name="rec")
                    nc.vector.reciprocal(rec, pt[:, 64:65])
                    nc.vector.tensor_scalar_mul(
                        out_sb[:, t, csl], pt[:, :64], rec)
                nc.sync.dma_start(
                    out_d.rearrange("(t p) c -> p t c", p=128)[
                        :, ic * 4:(ic + 1) * 4, csl],
                    out_sb[:, ic * 4:(ic + 1) * 4, csl])

            def local_prep(head, wp):
                hh = head % 2
                rs = slice(hh * 64, (hh + 1) * 64)
                lqT = projT["wlq"][rs]
                lk1T = projT["wlk1"][rs]

                # lk1 natural [s, d] via transposes (fp32: score chain)
                lk1nat = wp.tile([128, SC, 64], f32, tag="lk1nat",
                                 name="lk1nat", bufs=2)
                for t in range(SC):
                    pt = ps_tr.tile([128, 128], f32, tag="tr")
                    nc.tensor.transpose(
                        pt[:, :64], lk1T[:, t * 128:(t + 1) * 128],
                        ident[rs, rs])
                    nc.any.tensor_copy(lk1nat[:, t], pt[:, :64])
                # M = lk1^T @ lk1 [64, 64] (symmetric), fp32
                mps = ps_mm.tile([128, 512], f32, tag="mm", name="mps")
                for t in range(SC):
                    nc.tensor.matmul(mps[:64, :64], lhsT=lk1nat[:, t],
                                     rhs=lk1nat[:, t],
                                     start=(t == 0), stop=(t == SC - 1))
                # m_sb lives at the same base partition as lqT so the
                # qaug matmul has matching operand bases
                m_sb = wp.tile([128, 64], f32r, tag="m_sb", name="m_sb",
                               bufs=2)
                nc.any.tensor_copy(m_sb[rs], mps[:64, :64])
                # qaug rows 0:64 = (lq @ M)^T = M @ lq^T (M symmetric);
                # row 64 filled later with -max
                qaug = wp.tile([65, S], f32r, tag="qaug", name="qaug",
                               bufs=2)
                for ic in range(4):
                    mm = ps_mm.tile([128, 512], f32, tag="mm", name="mm")
                    nc.tensor.matmul(mm[:64], lhsT=m_sb[rs],
                                     rhs=lqT[:, ic * 512:(ic + 1) * 512],
                                     start=True, stop=True)
                    nc.any.tensor_copy(qaug[:64, ic * 512:(ic + 1) * 512],
                                       mm[:64])
                vaug = build_vaug(projT["wlv"][rs], wp)
                return dict(qaug=qaug, k2aug=k2aug[hh], vaug=vaug)

            def local_pass1(head, hs, wp):
                # pass 1: untransposed s[i, j] blocks; row max via free-dim
                # reduce (independent ops; no serial DVE chain)
                qaug_r = hs["qaug"][:64]
                k2aug_r = hs["k2aug"][:64]
                maxneg = wp.tile([128, SC], f32r, tag="maxneg", name="maxneg",
                                 bufs=2)
                for t in range(SC):
                    pmax = wp.tile([128, 4], f32, tag="pmax", name="pmax",
                                   bufs=2)
                    for j4 in range(4):
                        st = ps_mm.tile([128, 512], f32, tag="mm", name="st1")
                        nc.tensor.matmul(
                            st, lhsT=qaug_r[:, t * 128:(t + 1) * 128],
                            rhs=k2aug_r[:, j4 * 512:(j4 + 1) * 512],
                            start=True, stop=True)
                        nc.vector.tensor_reduce(pmax[:, j4:j4 + 1], st,
                                                axis=AX.X, op=ALU.max)
                    nc.vector.tensor_reduce(maxneg[:, t:t + 1], pmax,
                                            axis=AX.X, op=ALU.max,
                                            negate=True)
                mscr = dramp.tile([S], f32r, tag="mscr", name="mscr")
                nc.sync.dma_start(
                    mscr.rearrange("(t p) -> p t", p=128), maxneg)
                nc.sync.dma_start(hs["qaug"][64:65, :], mscr[None, :])

            # ---------- phase A: hidden^T, projections, global heads ----
            with (
                tc.tile_pool(name="hidT", bufs=1) as hp,
                tc.tile_pool(name="io", bufs=2) as iop,
            ):
                hidT = hp.tile([128, HC, S], f32r, name="hidT")
                hid_r = hid_d.rearrange("(c p) s -> p c s", p=128)
                # qkv weights first on the gpsimd queue so the first
                # projection matmuls start as soon as the first s-slice
                # of hidT lands
                wsb_g = {}
                for n in ["wq", "wk", "wv"]:
                    wsb_g[n] = iop.tile([128, HC, 128], f32r, tag="wg",
                                        name=f"w_{n}")
                    nc.gpsimd.dma_start(
                        wsb_g[n], w_d[n].rearrange("(c p) m -> p c m", p=128))
                # hid arrives in s-major slices so the wq projection can
                # chase the DMA instead of waiting for the full 8MB
                for icg in range(4):
                    isl = slice(icg * 512, (icg + 1) * 512)
                    for hc in range(HC):
                        eng = nc.sync if hc % 2 == 0 else nc.gpsimd
                        eng.dma_start(hidT[:, hc, isl], hid_r[:, hc, isl])
                for n in ["wq", "wk", "wv"]:
                    for half in range(2):
                        emit_proj_half(n, wsb_g[n], hidT, half)
                gvaug = {}
                for hh in range(2):
                    rs = slice(hh * 64, (hh + 1) * 64)
                    gvaug[hh] = build_vaug(projT["wv"][rs], wp_g)
                # interleave: global-head attention units between local
                # projection halves so the in-order PE queue always has
                # independent matmuls (keeps HAM warm)
                lp = [(n, half) for n in ["wlq", "wlk1", "wlk2", "wlv"]
                      for half in range(2)]
                wsbs = {}
                for i, (hh, ic) in enumerate(
                        [(h, c) for h in range(2) for c in range(4)]):
                    rs = slice(hh * 64, (hh + 1) * 64)
                    attention_ic(hh, projT["wk"][rs], projT["wq"][rs],
                                 gvaug[hh], False, ic)
                    n, half = lp[i]
                    if half == 0:
                        wsbs[n] = emit_wdma(n, iop)
                    emit_proj_half(n, wsbs[n], hidT, half)

            # ---------- phase B: local heads (stage-interleaved so
            # the PE never idles long enough to go HAM-cold) ----------
            with tc.tile_pool(name="wp_l", bufs=1) as wp_l:
                st2 = local_prep(2, wp_l)
                st3 = local_prep(3, wp_l)
                local_pass1(2, st2, wp_l)
                local_pass1(3, st3, wp_l)
                for ic in range(4):
                    attention_ic(2, st2["k2aug"], st2["qaug"], st2["vaug"],
                                 True, ic)
                for ic in range(4):
                    attention_ic(3, st3["k2aug"], st3["qaug"], st3["vaug"],
                                 True, ic)

    nc.compile()
    return nc


def _patch_ldw_opt():
    # walrus ships with the LDWEIGHTS optimizer disabled; fp32/f32r
    # matmuls pay a bundled weight reload per matmul, so enable the
    # optimizer (validated on HW: bit-identical output).
    from concourse import bass_utils
    if getattr(bass_utils, "_ldw_patched", False):
        return
    orig = bass_utils.bir_verify_and_optimise

    def patched(*a, **k):
        orig_run = bass_utils.run_command

        def run2(cmd, **kw):
            cmd = [c.replace("--enable-ldw-opt=false",
                             "--enable-ldw-opt=true") for c in cmd]
            return orig_run(cmd, **kw)

        bass_utils.run_command = run2
        try:
            return orig(*a, **k)
        finally:
            bass_utils.run_command = orig_run

    bass_utils.bir_verify_and_optimise = patched
    bass_utils._ldw_patched = True


def kernel(**inputs):
    from concourse import bass_utils

    if os.environ.get("LDW_OPT", "0") == "1":
        _patch_ldw_opt()

    global LAST_RESULTS
    if "nc" not in _CACHE:
        _CACHE["nc"] = _build()
    nc = _CACHE["nc"]

    inputs = dict(inputs)
    inputs["wlv"] = np.asarray(inputs["wlv1"]) + np.asarray(inputs["wlv2"])
    inputs["blv"] = np.asarray(inputs["blv1"]) + np.asarray(inputs["blv2"])
    hs = np.ascontiguousarray(np.asarray(inputs["hidden_states"], np.float32))
    am = np.ascontiguousarray(np.asarray(inputs["attention_mask"], np.float32))
    in_maps = []
    for c in range(N_CORES):
        b, g = c // 4, c % 4
        csl = slice(128 * g, 128 * (g + 1))
        m = {"hid": np.ascontiguousarray(hs[b].T), "mask": am[b, 0, 0]}
        for n in W_NAMES:
            m[n] = np.ascontiguousarray(
                np.asarray(inputs[n], np.float32)[:, csl])
            m["b" + n[1:]] = np.ascontiguousarray(
                np.asarray(inputs["b" + n[1:]], np.float32)[csl])
        in_maps.append(m)

    res = bass_utils.run_bass_kernel_spmd(
        nc, in_maps, list(range(N_CORES)),
        tmpdir=os.environ.get("BASS_TMPDIR"))
    LAST_RESULTS = res

    out = np.zeros((B, S, HID), np.float32)
    for c in range(N_CORES):
        b, g = c // 4, c % 4
        o = res.results[c]["out"]
        out[b, :, 128 * g:128 * (g + 1)] = o[:, :128]
        out[b, :, 512 + 128 * g:512 + 128 * (g + 1)] = o[:, 128:]
    return out
